# revision 12
# baseline (speedup 1.0000x reference)
"""GSN (ChebConv-style GNN) Trainium2 kernel for nn_GSN_14783277433402.

Math (K=3, derived from the reference):
  per layer: out = relu( x@(w0+w1-w2) + 2*(S@x)@w2 + b + Asrc@ew.sum(0) )
  where S[dst,src] += norm[src]*norm[dst]  (norm = outdeg^-1/2), and
  Asrc = segment_sum(edge_attr, src).  Then sorted-batch mean pool, linear
  head, log_softmax.

Distribution: dst-node parallel over 8 NeuronCores.  Host assigns each node
to one of 392 dst-tiles (49 tiles x 128 rows per core, load balanced by
in-degree).  Per layer each core:
  1. computes Z = diag(norm) * X @ (2*w[2]) for its rows, all-gathers Z
     (bf16 [50176,128]) so every core holds the full projected matrix,
  2. gathers Z[src] rows for its in-edges with dma_gather (256B rows),
  3. segment-sums them per dst tile as one-hot matmuls on the TensorEngine
     (P[p,d] = (dloc[p]==d), built on VectorE from an iota + is_equal),
  4. adds the dense terms X@Wc + [Asrc|1]@[ew;b] (accumulated in PSUM via
     matmuls with the feature-major X as stationary operand) and applies
     norm scaling + relu on VectorE.
Edges are bucketed per tile into "low" (src row < 32768) and "high" chunks
because dma_gather indices are int16.  Pooling partials are one more one-hot
matmul; the tiny mean/linear/log_softmax epilogue runs on host.
"""
import sys
import numpy as np

if "/opt/trn_rl_repo" not in sys.path:
    sys.path.insert(0, "/opt/trn_rl_repo")

import ml_dtypes

import concourse.bass as bass
import concourse.bacc as bacc
import concourse.mybir as mybir
import concourse.tile as tile
from concourse import bass_utils

BF16 = mybir.dt.bfloat16
F32 = mybir.dt.float32
I16 = mybir.dt.int16
NPBF16 = ml_dtypes.bfloat16

N, E, G, H, FN, FE, CLS = 50000, 800000, 64, 128, 9, 4, 4
NC = 8                 # cores
TPC = 49               # dst tiles per core
NT = NC * TPC          # 392 global tiles
ROWS = NT * 128        # 50176 padded rows
LOWROWS = 32768        # int16 index boundary
WT = 7                 # tiles per gather window
NW = TPC // WT         # windows per core

_COMPILED = {}


# ----------------------------------------------------------------------------
# host preprocessing
# ----------------------------------------------------------------------------
def _host_prep(x, edge_attr, w0, ew0, b0, w1, ew1, b1, edge_index, batch):
    src = np.asarray(edge_index[0]).astype(np.int64)
    dst = np.asarray(edge_index[1]).astype(np.int64)
    x = np.asarray(x, np.float32)
    edge_attr = np.asarray(edge_attr, np.float32)
    batch = np.asarray(batch).astype(np.int64)

    deg = np.bincount(src, minlength=N).astype(np.float32)
    norm = np.where(deg > 0, deg ** -0.5, 0.0).astype(np.float32)
    asrc = np.stack(
        [np.bincount(src, weights=edge_attr[:, j], minlength=N) for j in range(FE)],
        axis=1,
    ).astype(np.float32)

    # node -> (tile, rank): deal nodes sorted by in-degree (desc) round-robin
    indeg = np.bincount(dst, minlength=N)
    order = np.argsort(-indeg, kind="stable")
    tile_of = np.empty(N, np.int64)
    rank_of = np.empty(N, np.int64)
    tile_of[order] = np.arange(N) % NT
    rank_of[order] = np.arange(N) // NT
    pi = tile_of * 128 + rank_of                  # node -> global row

    # per-edge tile / class / rank
    te = tile_of[dst]
    pis = pi[src]
    cls = (pis >= LOWROWS).astype(np.int64)       # 0 = low, 1 = high
    ordE = np.lexsort((pis, cls, te))
    te_s, cls_s, pis_s = te[ordE], cls[ordE], pis[ordE]
    dloc_s = rank_of[dst][ordE]

    # group = tile*2 + cls ; ranks within group
    gkey = te_s * 2 + cls_s
    gcnt = np.bincount(gkey, minlength=NT * 2)
    gstart = np.concatenate(([0], np.cumsum(gcnt)[:-1]))
    r = np.arange(E) - gstart[gkey]

    cnt_lo = gcnt[0::2]
    cnt_hi = gcnt[1::2]
    lowc = max(1, int(np.ceil(cnt_lo.max() / 128)))
    highc = max(1, int(np.ceil(cnt_hi.max() / 128)))
    cht = lowc + highc

    # slot arrays (global, tile-major; per-core slices are contiguous)
    idxlo = np.zeros(NT * lowc * 128, np.int16)
    idxhi = np.zeros(NT * highc * 128, np.int16)
    dloc = np.full(NT * cht * 128, 255.0, np.float32)

    lo = cls_s == 0
    hi = ~lo
    jl = te_s[lo] * lowc * 128 + r[lo]
    jh = te_s[hi] * highc * 128 + r[hi]
    idxlo[jl] = pis_s[lo].astype(np.int16)
    idxhi[jh] = (pis_s[hi] - LOWROWS).astype(np.int16)
    cl, pl = r[lo] // 128, r[lo] % 128
    ch, ph = r[hi] // 128, r[hi] % 128
    dloc[(te_s[lo] * cht + cl) * 128 + pl] = dloc_s[lo]
    dloc[(te_s[hi] * cht + lowc + ch) * 128 + ph] = dloc_s[hi]

    # per-row (padded) node tables
    inv = np.full(ROWS, -1, np.int64)
    inv[pi] = np.arange(N)
    occ = inv >= 0
    invc = np.where(occ, inv, 0)

    def rowtab(vals_n, fill=0.0):
        out = np.where(occ, vals_n[invc], fill)
        return out

    xT = np.zeros((FN, ROWS), np.float32)
    xT[:, occ] = x[inv[occ]].T
    a5 = np.zeros((5, ROWS), np.float32)
    a5[:FE, occ] = asrc[inv[occ]].T
    a5[FE, occ] = 1.0
    normr = rowtab(norm).astype(np.float32)       # [ROWS]
    boneg = np.where(occ, batch[invc], -1)

    # weights (host-combined, f32 math then bf16)
    w0 = np.asarray(w0, np.float32); w1 = np.asarray(w1, np.float32)
    ew0 = np.asarray(ew0, np.float32); ew1 = np.asarray(ew1, np.float32)
    w0c = w0[0] + w0[1] - w0[2]
    w1c = w1[0] + w1[1] - w1[2]
    z0w = 2.0 * w0[2]
    z1w = 2.0 * w1[2]
    e0w = np.concatenate([ew0.sum(0), np.asarray(b0, np.float32)[None, :]], axis=0)
    e1w = np.concatenate([ew1.sum(0), np.asarray(b1, np.float32)[None, :]], axis=0)

    def wrap_idx(a):
        # [n] -> [128, n/16]; unwrap rule idx[j] = ap[j%16, j//16], 8x replicated
        t16 = a.reshape(-1, 16).T
        return np.tile(t16, (8, 1)).copy()

    in_maps = []
    for c in range(NC):
        tslice = slice(c * TPC, (c + 1) * TPC)
        rslice = slice(c * TPC * 128, (c + 1) * TPC * 128)
        nr = normr[rslice].reshape(TPC, 128).T
        bg = boneg[rslice].reshape(TPC, 128).T    # [128, TPC]
        bone = np.zeros((128, TPC * G), NPBF16)
        pp, tt = np.nonzero(bg >= 0)
        bone[pp, tt * G + bg[pp, tt]] = 1.0
        in_maps.append({
            "xT": xT[:, rslice].astype(NPBF16),
            "a5T": a5[:, rslice].astype(NPBF16),
            "normQ": np.ascontiguousarray(nr),
            "dloc": np.ascontiguousarray(
                dloc[c * TPC * cht * 128:(c + 1) * TPC * cht * 128]
                .reshape(TPC * cht, 128).T.astype(NPBF16)),
            "idxlo": wrap_idx(idxlo[c * TPC * lowc * 128:(c + 1) * TPC * lowc * 128]),
            "idxhi": wrap_idx(idxhi[c * TPC * highc * 128:(c + 1) * TPC * highc * 128]),
            "bone": bone,
            "w0c": w0c.astype(NPBF16), "z0w": z0w.astype(NPBF16),
            "e0w": e0w.astype(NPBF16),
            "w1c": w1c.astype(NPBF16), "z1w": z1w.astype(NPBF16),
            "e1w": e1w.astype(NPBF16),
        })

    counts = np.bincount(batch, minlength=G).astype(np.float32)
    return in_maps, lowc, highc, counts


# ----------------------------------------------------------------------------
# device program
# ----------------------------------------------------------------------------
def _build(lowc, highc, stage=4):
    # stage: 1=Z+collective only, 2=+gathers, 3=+edge/dense/combine, 4=full
    cht = lowc + highc
    nc = bacc.Bacc("TRN2", target_bir_lowering=False, debug=False, num_devices=NC)

    din = {}
    for name, shape, dt in [
        ("xT", [FN, TPC * 128], BF16),
        ("a5T", [5, TPC * 128], BF16),
        ("normQ", [128, TPC], F32),
        ("dloc", [128, TPC * cht], BF16),
        ("idxlo", [128, TPC * lowc * 8], I16),
        ("idxhi", [128, TPC * highc * 8], I16),
        ("bone", [128, TPC * G], BF16),
        ("w0c", [FN, H], BF16), ("z0w", [FN, H], BF16), ("e0w", [5, H], BF16),
        ("w1c", [H, H], BF16), ("z1w", [H, H], BF16), ("e1w", [5, H], BF16),
    ]:
        din[name] = nc.dram_tensor(name, shape, dt, kind="ExternalInput")
    pool_out = nc.dram_tensor("pool_out", [G, H], F32, kind="ExternalOutput")

    zloc = [nc.dram_tensor(f"z{l}loc", [TPC * 128, H], BF16) for l in range(2)]
    zfull = [nc.dram_tensor(f"z{l}full", [ROWS, H], BF16, addr_space="Shared")
             for l in range(2)]

    with tile.TileContext(nc) as tc:
        with (
            tc.tile_pool(name="res", bufs=1) as res,
            tc.tile_pool(name="ylo", bufs=2) as ylop,
            tc.tile_pool(name="yhi", bufs=2) as yhip,
            tc.tile_pool(name="pt", bufs=3) as ptp,
            tc.tile_pool(name="work", bufs=4) as wk,
            tc.tile_pool(name="zeps", bufs=1, space="PSUM") as zps,
            tc.tile_pool(name="eps", bufs=2, space="PSUM") as eps,
            tc.tile_pool(name="dps", bufs=2, space="PSUM") as dps,
            tc.tile_pool(name="trps", bufs=1, space="PSUM") as trps,
            tc.tile_pool(name="pps", bufs=1, space="PSUM") as pps,
        ):
            # resident loads
            def load(name, shape, dt):
                t = res.tile(shape, dt, tag=name)
                nc.sync.dma_start(t[:], din[name].ap())
                return t
            xT = load("xT", [FN, TPC * 128], BF16)
            a5T = load("a5T", [5, TPC * 128], BF16)
            normQ = load("normQ", [128, TPC], F32)
            dlocT = load("dloc", [128, TPC * cht], BF16)
            idxlo = load("idxlo", [128, TPC * lowc * 8], I16)
            idxhi = load("idxhi", [128, TPC * highc * 8], I16)
            bone = load("bone", [128, TPC * G], BF16)
            w0c = load("w0c", [FN, H], BF16)
            z0w = load("z0w", [FN, H], BF16)
            e0w = load("e0w", [5, H], BF16)
            w1c = load("w1c", [H, H], BF16)
            z1w = load("z1w", [H, H], BF16)
            e1w = load("e1w", [5, H], BF16)

            iota = res.tile([128, cht, 128], BF16, tag="iota")
            nc.gpsimd.iota(iota[:], pattern=[[0, cht], [1, 128]], base=0,
                           channel_multiplier=0,
                           allow_small_or_imprecise_dtypes=True)
            iodiag = res.tile([128, 128], BF16, tag="iodiag")
            nc.gpsimd.iota(iodiag[:], pattern=[[1, 128]], base=0,
                           channel_multiplier=-1,
                           allow_small_or_imprecise_dtypes=True)
            ident = res.tile([128, 128], BF16, tag="ident")
            nc.vector.tensor_scalar(ident[:], iodiag[:], 0.0, None,
                                    op0=mybir.AluOpType.is_equal)

            h1f = res.tile([128, TPC * 128], BF16, tag="h1f")
            if stage < 4:
                nc.vector.memset(h1f[:], 0.0)

            ts = bass.ts

            def layer(l):
                xlhs = xT if l == 0 else h1f          # [K, TPC*128] f-major
                kdim = FN if l == 0 else H
                wc = w0c if l == 0 else w1c
                zw = z0w if l == 0 else z1w
                ew = e0w if l == 0 else e1w

                # --- Z projection + all-gather ---
                for t in range(TPC):
                    zp = zps.tile([128, H], F32, tag="zp")
                    nc.tensor.matmul(zp[:], xlhs[:kdim, ts(t, 128)], zw[:],
                                     start=True, stop=True)
                    zs = wk.tile([128, H], BF16, tag="zs")
                    nc.vector.tensor_scalar(zs[:], zp[:], normQ[:, t:t + 1], None,
                                            op0=mybir.AluOpType.mult)
                    nc.sync.dma_start(zloc[l].ap()[ts(t, 128), :], zs[:])
                nc.gpsimd.collective_compute(
                    "AllGather", mybir.AluOpType.bypass,
                    replica_groups=[list(range(NC))],
                    ins=[zloc[l].ap().opt()], outs=[zfull[l].ap().opt()],
                )
                if stage <= 1:
                    return

                # --- windows: gather + per-tile compute ---
                for w in range(NW):
                    nlo = WT * lowc * 128
                    nhi = WT * highc * 128
                    ylo = ylop.tile([128, WT * lowc, H], BF16, tag="ylo")
                    nc.gpsimd.dma_gather(
                        ylo[:], zfull[l].ap(),
                        idxlo[:, w * WT * lowc * 8:(w + 1) * WT * lowc * 8],
                        num_idxs=nlo, num_idxs_reg=nlo, elem_size=H,
                        single_packet=False)
                    yhi = yhip.tile([128, WT * highc, H], BF16, tag="yhi")
                    nc.gpsimd.dma_gather(
                        yhi[:], zfull[l].ap()[LOWROWS:, :],
                        idxhi[:, w * WT * highc * 8:(w + 1) * WT * highc * 8],
                        num_idxs=nhi, num_idxs_reg=nhi, elem_size=H,
                        single_packet=False)

                    if stage <= 2:
                        dmy = wk.tile([128, H], BF16, tag="hb")
                        nc.vector.tensor_tensor(dmy[:], ylo[:, 0, :],
                                                yhi[:, 0, :],
                                                op=mybir.AluOpType.add)
                        continue
                    for tw in range(WT):
                        t = w * WT + tw
                        # dense terms -> PSUM
                        dp = dps.tile([128, H], F32, tag="dp")
                        nc.tensor.matmul(dp[:], xlhs[:kdim, ts(t, 128)], wc[:],
                                         start=True, stop=False)
                        nc.tensor.matmul(dp[:], a5T[:, ts(t, 128)], ew[:],
                                         start=False, stop=True)
                        # one-hot P for this tile
                        pt = ptp.tile([128, cht, 128], BF16, tag="pt")
                        dl = dlocT[:, t * cht:(t + 1) * cht].unsqueeze(-1) \
                            .broadcast_to((128, cht, 128))
                        nc.vector.tensor_tensor(pt[:], iota[:], dl,
                                                op=mybir.AluOpType.is_equal)
                        # edge segment-sum matmuls
                        ep = eps.tile([128, H], F32, tag="ep")
                        for c in range(lowc):
                            nc.tensor.matmul(ep[:], pt[:, c, :],
                                             ylo[:, tw * lowc + c, :],
                                             start=(c == 0), stop=False)
                        for c in range(highc):
                            nc.tensor.matmul(ep[:], pt[:, lowc + c, :],
                                             yhi[:, tw * highc + c, :],
                                             start=False, stop=(c == highc - 1))
                        # combine: h = relu(dense + 2*norm_dst*edge)
                        es = wk.tile([128, H], F32, tag="es")
                        nc.vector.tensor_scalar(es[:], ep[:], normQ[:, t:t + 1],
                                                None, op0=mybir.AluOpType.mult)
                        hs = wk.tile([128, H], F32, tag="hs")
                        nc.vector.tensor_tensor(hs[:], es[:], dp[:],
                                                op=mybir.AluOpType.add)
                        hb = wk.tile([128, H], BF16, tag="hb")
                        nc.vector.tensor_scalar(hb[:], hs[:], 0.0, None,
                                                op0=mybir.AluOpType.max)
                        if stage <= 3:
                            continue
                        if l == 0:
                            # transpose into resident f-major h1 (PE + identity)
                            trp = trps.tile([128, H], BF16, tag="trp")
                            nc.tensor.transpose(trp[:], hb[:], ident[:])
                            nc.vector.tensor_copy(h1f[:, ts(t, 128)], trp[:])
                        else:
                            nc.tensor.matmul(poolp[:], bone[:, t * G:(t + 1) * G],
                                             hb[:], start=(t == 0),
                                             stop=(t == TPC - 1))

            layer(0)
            poolp = pps.tile([G, H], F32, tag="poolp")
            layer(1)
            po = wk.tile([G, H], F32, tag="po")
            if stage >= 4:
                nc.vector.tensor_copy(po[:], poolp[:])
            else:
                nc.vector.memset(po[:], 0.0)
            nc.sync.dma_start(pool_out.ap(), po[:])

    nc.compile()
    return nc


# ----------------------------------------------------------------------------
# entry point
# ----------------------------------------------------------------------------
def kernel(x, edge_attr, w0, ew0, b0, w1, ew1, b1, lin_w, lin_b, edge_index, batch):
    in_maps, lowc, highc, counts = _host_prep(
        x, edge_attr, w0, ew0, b0, w1, ew1, b1, edge_index, batch)

    key = (lowc, highc)
    if key not in _COMPILED:
        _COMPILED[key] = _build(lowc, highc)
    nc = _COMPILED[key]

    res = bass_utils.run_bass_kernel_spmd(nc, in_maps, core_ids=list(range(NC)))
    pooled = np.zeros((G, H), np.float32)
    for c in range(NC):
        pooled += np.asarray(res.results[c]["pool_out"], np.float32)
    pooled /= np.maximum(counts, 1.0)[:, None]

    logits = pooled @ np.asarray(lin_w, np.float32) + np.asarray(lin_b, np.float32)
    z = logits - logits.max(axis=1, keepdims=True)
    lse = np.log(np.exp(z).sum(axis=1, keepdims=True))
    return (z - lse).astype(np.float32)
